# revision 14
# baseline (speedup 1.0000x reference)
"""Single-head causal attention (B=4, T=2048, D=1024, H=64) on 8 TRN2 cores.

Sharding: core = (batch b, group g). Each core owns the interleaved half of
the query blocks of one batch (g=0: even 128-row blocks, g=1: odd), arranged
"mine first, partner second" via a host-side row permutation so the causal
structure (and hence the instruction stream) is identical on all 8 cores.

v2 structure (vs the v1 baseline):
- Wk and 8*Wq are packed into one [128]-wide stationary operand, so each
  projection pass computes k (partitions 0:63) and q (64:127) together:
  4 passes/chunk (v, hi*xh, lo*xh, hi*xl) instead of 7.
- Scores in 2 matmul passes instead of 3: pass 1 contracts 128 partitions
  with lhsT=[q_lo; q_hi] stacked and rhs=[k_hi; k_hi] duplicated
  (= q_hi*k_hi + q_lo*k_hi), pass 2 adds q_hi*k_lo (64-contraction).
- Per-block AV + finalize pipelined behind each score block (no monolithic
  AV tail), q/k split peeled per 512-col region so early blocks start while
  late regions still project, junk matmuls warm the PE during the x DMA.
- PSUM: 5-slot rotation shared by qk-region accumulators and score chunks
  (5 banks) + two-column-pass v (2 banks) + av/tr rotation (1 bank).
"""

import numpy as np
import ml_dtypes

import concourse.bass as bass
import concourse.bacc as bacc
import concourse.tile as tile
import concourse.mybir as mybir
from concourse.bass_utils import run_bass_kernel_spmd
from concourse.masks import make_identity

BF16 = mybir.dt.bfloat16
F32 = mybir.dt.float32
BFNP = ml_dtypes.bfloat16

B, T, D, H, P = 4, 2048, 1024, 64, 128
NB = T // P       # 16 key tiles (128 rows each) per batch
NQ = NB // 2      # 8 local query blocks per core
DC = D // P       # 8 contraction chunks
KC = 512          # matmul moving-dim chunk (one PSUM bank of fp32)
NEG = -1.0e30
ACT = mybir.ActivationFunctionType
ADD = mybir.AluOpType.add
SUB = mybir.AluOpType.subtract
MAX = mybir.AluOpType.max


def build_nc():
    nc = bacc.Bacc("TRN2", target_bir_lowering=False, debug=False,
                   num_devices=8)
    xt_hi = nc.dram_tensor("xt_hi", [D, T], BF16, kind="ExternalInput")
    xt_lo = nc.dram_tensor("xt_lo", [D, T], BF16, kind="ExternalInput")
    # weights pre-arranged on host to the SBUF layout [P, DC*{128,64}];
    # wkq = [Wk | 8*Wq] so one matmul emits k on partitions 0:63, q on 64:127
    wkq_hi = nc.dram_tensor("wkq_hi", [P, DC * P], BF16, kind="ExternalInput")
    wkq_lo = nc.dram_tensor("wkq_lo", [P, DC * P], BF16, kind="ExternalInput")
    wv = nc.dram_tensor("wv", [P, DC * H], BF16, kind="ExternalInput")
    bqk = nc.dram_tensor("bqk", [P, 1], F32, kind="ExternalInput")
    bvb = nc.dram_tensor("bvb", [H, 1], F32, kind="ExternalInput")
    maska = nc.dram_tensor("maska", [P, P], F32, kind="ExternalInput")
    maskb = nc.dram_tensor("maskb", [P, P], F32, kind="ExternalInput")
    out = nc.dram_tensor("out", [NQ * P, H], F32, kind="ExternalOutput")

    with tile.TileContext(nc) as tc:
        with (
            tc.tile_pool(name="singles", bufs=1) as singles,
            tc.tile_pool(name="probs", bufs=3) as probs_pool,
            tc.tile_pool(name="stats", bufs=6) as stats,
        ):
            # ---- persistent SBUF ----
            s_xth = singles.tile([P, DC, T], BF16)
            s_xtl = singles.tile([P, DC, T], BF16)
            s_wkqh = singles.tile([P, DC, P], BF16)
            s_wkql = singles.tile([P, DC, P], BF16)
            s_wv = singles.tile([P, DC, H], BF16)
            s_bqk = singles.tile([P, 1], F32)
            s_bv = singles.tile([H, 1], F32)
            s_ma = singles.tile([P, P], F32)
            s_mb = singles.tile([P, P], F32)

            # weights/bias/masks early on the scalar queue; x chunks follow
            # in consumption order, split per half-chunk so the first
            # projection matmuls start ~1us in: hi on sync, lo on scalar.
            for s_w, d_w in ((s_wv, wv), (s_wkqh, wkq_hi), (s_wkql, wkq_lo)):
                nc.scalar.dma_start(
                    s_w[:, :, :].rearrange("p c h -> p (c h)"), d_w[:, :])
            nc.scalar.dma_start(s_bqk[:, :], bqk[:, :])
            nc.scalar.dma_start(s_bv[:, :], bvb[:, :])
            nc.scalar.dma_start(s_ma[:, :], maska[:, :])
            nc.scalar.dma_start(s_mb[:, :], maskb[:, :])
            TH = T // 2
            for c in range(DC):
                for h in range(2):
                    nc.sync.dma_start(
                        s_xth[:, c, h * TH:(h + 1) * TH],
                        xt_hi[c * P:(c + 1) * P, h * TH:(h + 1) * TH])
                    nc.scalar.dma_start(
                        s_xtl[:, c, h * TH:(h + 1) * TH],
                        xt_lo[c * P:(c + 1) * P, h * TH:(h + 1) * TH])

            s_v = singles.tile([H, T], BF16)
            s_vnat = singles.tile([P, NB, H], BF16)
            s_f = singles.tile([P, T], F32)       # split staging, per region
            s_k2 = singles.tile([P, T], BF16)     # [k_hi; k_hi] duplicated
            s_kl = singles.tile([P, T], BF16)     # k_lo on partitions 64:127
            s_q2 = singles.tile([P, NQ * P], BF16)  # [q_lo; q_hi] stacked
            s_probsT = singles.tile([P, NB, NQ * P], BF16)
            s_z = singles.tile([P, NQ, 4], F32)
            nc.gpsimd.memset(s_z[:, :, :], 0.0)
            s_outT = singles.tile([H, NQ * P], F32)
            s_id = singles.tile([H, H], F32)
            make_identity(nc, s_id[:, :])
            s_zs = singles.tile([P, NQ], F32)
            s_rz = singles.tile([P, NQ], F32)
            s_on = singles.tile([P, NQ, H], F32)
            out3 = out[:, :].rearrange("(j p) h -> j p h", p=P)

            with (
                tc.tile_pool(name="work_psum", bufs=5, space="PSUM") as wk,
                tc.tile_pool(name="v_psum", bufs=1, space="PSUM") as vp,
                tc.tile_pool(name="av_psum", bufs=1, space="PSUM") as avp,
            ):
                # qk region accumulators take the first 4 of the 5 "w" slots;
                # score chunks (and the tail transposes) rotate through the
                # rest for the whole phase 2.
                qk_r = [wk.tile([P, KC], F32, tag="w", name=f"qk_r{r}")
                        for r in range(4)]
                v0 = vp.tile([H, KC], F32, tag="v")  # cols 0:512
                avps = avp.tile([H, NQ * P], F32, tag="av")

                # ---- phase 1: projections, chunks 0..DC-2 ----
                # per chunk: v cols 0:512, then [k|q]hi*xh, lo*xh, hi*xl
                for c in range(DC - 1):
                    nc.tensor.matmul(
                        v0[:, :], lhsT=s_wv[:, c, :],
                        rhs=s_xth[:, c, 0:KC],
                        start=(c == 0), stop=False)
                    for ip, (wpl, xpl) in enumerate(
                            ((s_wkqh, s_xth), (s_wkql, s_xth),
                             (s_wkqh, s_xtl))):
                        for r in range(4):
                            nc.tensor.matmul(
                                qk_r[r][:, :], lhsT=wpl[:, c, :],
                                rhs=xpl[:, c, r * KC:(r + 1) * KC],
                                start=(c == 0 and ip == 0), stop=False)

                # ---- peeled last chunk, per 512-col region + fused split ----
                c = DC - 1

                def peel_region(r):
                    n0, n1 = r * KC, (r + 1) * KC
                    if r == 0:
                        nc.tensor.matmul(v0[:, :], lhsT=s_wv[:, c, :],
                                         rhs=s_xth[:, c, 0:KC],
                                         start=False, stop=True)
                    for ip, (wpl, xpl) in enumerate(
                            ((s_wkqh, s_xth), (s_wkql, s_xth),
                             (s_wkqh, s_xtl))):
                        nc.tensor.matmul(qk_r[r][:, :], lhsT=wpl[:, c, :],
                                         rhs=xpl[:, c, n0:n1],
                                         start=False, stop=(ip == 2))
                    # split: f32 stage (+bias), k hi dup, k lo, q hi/lo stack
                    nc.scalar.activation(s_f[:, n0:n1], qk_r[r][:, :],
                                         ACT.Identity, bias=s_bqk[:, 0:1],
                                         scale=1.0)
                    if r == 0:
                        nc.scalar.copy(s_v[:, 0:KC], v0[:, :])
                        nc.sync.dma_start(s_vnat[:, 0:4, :], s_v[:, 0:KC],
                                          transpose=True)
                    nc.scalar.copy(s_k2[0:H, n0:n1], s_f[0:H, n0:n1])
                    nc.vector.tensor_scalar_mul(s_k2[H:P, n0:n1],
                                                s_f[0:H, n0:n1], 1.0)
                    nc.vector.tensor_tensor(s_kl[H:P, n0:n1], s_f[0:H, n0:n1],
                                            s_k2[0:H, n0:n1], SUB)
                    if r < 2:  # q lives in cols 0:1024 (partitions 64:127)
                        nc.scalar.copy(s_q2[H:P, n0:n1], s_f[H:P, n0:n1])
                        nc.vector.tensor_tensor(s_q2[0:H, n0:n1],
                                                s_f[H:P, n0:n1],
                                                s_q2[H:P, n0:n1], SUB)

                def score_block(i):
                    """scores+softmax+transpose for local q block i."""
                    K = P * (i + 1)
                    q0, q1 = i * P, (i + 1) * P
                    nch = (K + KC - 1) // KC
                    mx4 = stats.tile([P, 4], F32, tag="mx")
                    chunks = []
                    for pi, (koff, msk) in enumerate(((0, s_ma),
                                                      (NQ * P, s_mb))):
                        for n0 in range(0, K, KC):
                            nn = min(KC, K - n0)
                            sp = wk.tile([P, KC], F32, tag="w", name="sp")
                            # pass 1: [q_lo;q_hi].T @ [k_hi;k_hi]
                            nc.tensor.matmul(
                                sp[:, 0:nn], lhsT=s_q2[:, q0:q1],
                                rhs=s_k2[:, koff + n0:koff + n0 + nn],
                                start=True, stop=False)
                            # pass 2: q_hi.T @ k_lo (rows 64:127)
                            nc.tensor.matmul(
                                sp[:, 0:nn], lhsT=s_q2[H:P, q0:q1],
                                rhs=s_kl[H:P, koff + n0:koff + n0 + nn],
                                start=False, stop=True)
                            if n0 + nn == K:  # mask rides in the last chunk
                                nc.vector.tensor_tensor(
                                    sp[:, nn - P:nn], sp[:, nn - P:nn],
                                    msk[:, :], ADD)
                            ci = pi * nch + n0 // KC
                            nc.vector.tensor_reduce(
                                mx4[:, ci:ci + 1], sp[:, 0:nn],
                                axis=mybir.AxisListType.X, op=MAX)
                            chunks.append((sp, pi, n0, nn, ci))
                    negm = stats.tile([P, 1], F32, tag="negm")
                    nc.vector.tensor_reduce(negm[:, :], mx4[:, 0:2 * nch],
                                            axis=mybir.AxisListType.X,
                                            op=MAX, negate=True)
                    probs = probs_pool.tile([P, T], BF16)
                    for sp, pi, n0, nn, ci in chunks:
                        po = pi * K + n0
                        nc.scalar.activation(probs[:, po:po + nn], sp[:, 0:nn],
                                             ACT.Exp, bias=negm[:, :],
                                             scale=1.0,
                                             accum_out=s_z[:, i, ci:ci + 1])
                    # one combined transpose per block (parts a+b), on sync:
                    # probsT middle index ti enumerates [a0..ai, b0..bi]
                    nc.sync.dma_start(s_probsT[:, 0:2 * (i + 1), q0:q1],
                                      probs[:, 0:2 * K], transpose=True)

                def av_block(i):
                    """AV accumulation for local q block i (no finalize)."""
                    q0, q1 = i * P, (i + 1) * P
                    tiles = list(range(0, i + 1)) + list(range(NQ, NQ + i + 1))
                    for ti, t in enumerate(tiles):
                        nc.tensor.matmul(
                            avps[:, q0:q1], lhsT=s_vnat[:, t, :],
                            rhs=s_probsT[:, ti, q0:q1],
                            start=(ti == 0), stop=(ti == len(tiles) - 1))

                def v1_region(r):
                    """v columns of region r in {1,2,3}, all chunks; copy out
                    and transpose that region to natural layout."""
                    n0, n1 = r * KC, (r + 1) * KC
                    vr = vp.tile([H, KC], F32, tag="v", name=f"v_r{r}")
                    for cc in range(DC):
                        nc.tensor.matmul(
                            vr[:, :], lhsT=s_wv[:, cc, :],
                            rhs=s_xth[:, cc, n0:n1],
                            start=(cc == 0), stop=(cc == DC - 1))
                    nc.scalar.copy(s_v[:, n0:n1], vr[:, :])
                    nc.sync.dma_start(s_vnat[:, 4 * r:4 * r + 4, :],
                                      s_v[:, n0:n1], transpose=True)

                # ---- phase 2 schedule: peel regions / early blocks /
                # late blocks with per-block AV pipelined into the score
                # stat/exp bubbles; all finalize batched in the tail ----
                # NB: block i part a reads k regions 0..(K-1)//KC, part b
                # reads regions 2..2+(K-1)//KC, q block i reads region i//4;
                # av block i needs v regions 0..i//4 and 2..2+i//4.
                peel_region(0)
                peel_region(2)
                score_block(3)
                peel_region(1)
                score_block(2)
                peel_region(3)
                v1_region(1)
                score_block(1)
                v1_region(2)
                score_block(0)
                score_block(7)
                av_block(3)
                score_block(6)
                v1_region(3)
                av_block(2)
                av_block(1)
                score_block(5)
                av_block(0)
                score_block(4)
                av_block(7)
                av_block(6)
                av_block(5)
                av_block(4)

                # ---- tail: Z, reciprocal, +bv, transpose, scale, store ----
                nc.vector.tensor_reduce(s_zs[:, :], s_z[:, :, :],
                                        axis=mybir.AxisListType.X, op=ADD)
                nc.vector.reciprocal(s_rz[:, :], s_zs[:, :])
                nc.scalar.activation(s_outT[:, :], avps[:, :], ACT.Identity,
                                     bias=s_bv[:, 0:1], scale=1.0)
                for j in range(NQ):
                    tps = wk.tile([P, H], F32, tag="w", name="tps")
                    nc.tensor.transpose(tps[:, :], s_outT[:, j * P:(j + 1) * P],
                                        s_id[:, :])
                    nc.vector.tensor_scalar_mul(s_on[:, j, :], tps[:, :],
                                                s_rz[:, j:j + 1])
                    nc.gpsimd.dma_start(out3[j, :, :], s_on[:, j, :])
    nc.compile()
    return nc


_NC_CACHE = {}


def _get_nc():
    if "nc" not in _NC_CACHE:
        _NC_CACHE["nc"] = build_nc()
    return _NC_CACHE["nc"]


def _split_bf(a):
    hi = a.astype(BFNP)
    lo = (a - hi.astype(np.float32)).astype(BFNP)
    return hi, lo


def _w_layout(w):
    # [D, M] -> SBUF layout [P, DC*M] (chunk-major along free dim)
    m = w.shape[1]
    return np.ascontiguousarray(
        w.reshape(DC, P, m).transpose(1, 0, 2).reshape(P, DC * m))


LAST_RESULT = None


def kernel(x, Wq, bq, Wk, bk, Wv, bv, _trace=False, **_run_kwargs):
    global LAST_RESULT
    x = np.ascontiguousarray(np.asarray(x, dtype=np.float32))
    Wq = np.asarray(Wq, dtype=np.float32)
    Wk = np.asarray(Wk, dtype=np.float32)
    Wv = np.asarray(Wv, dtype=np.float32)
    bq = np.asarray(bq, dtype=np.float32)
    bk = np.asarray(bk, dtype=np.float32)
    bv = np.asarray(bv, dtype=np.float32)

    # pack [Wk | 8*Wq] -> [D, 128]; the 8x is the softmax scale sqrt(H)
    wkq = np.concatenate([Wk, 8.0 * Wq], axis=1)
    wkqh, wkql = _split_bf(wkq)
    wkqh, wkql = _w_layout(wkqh), _w_layout(wkql)
    wvh = _w_layout(Wv.astype(BFNP))
    bqk = np.ascontiguousarray(
        np.concatenate([bk, 8.0 * bq]).reshape(P, 1))
    bvb = np.ascontiguousarray(bv.reshape(H, 1))
    r = np.arange(P)
    maska = np.where(r[None, :] <= r[:, None], 0.0, NEG).astype(np.float32)
    mb_g0 = np.full((P, P), NEG, dtype=np.float32)
    mb_g1 = np.zeros((P, P), dtype=np.float32)

    in_maps = []
    perms = []
    for core in range(8):
        b, g = core // 2, core % 2
        mine = list(range(g, NB, 2))
        partner = list(range(1 - g, NB, 2))
        perm = np.concatenate(
            [np.arange(blk * P, (blk + 1) * P) for blk in mine + partner])
        perms.append(perm)
        xt = np.ascontiguousarray(x[b][perm].T)  # [D, T] fp32
        xth, xtl = _split_bf(xt)
        in_maps.append({
            "xt_hi": xth, "xt_lo": xtl,
            "wkq_hi": wkqh, "wkq_lo": wkql, "wv": wvh,
            "bqk": bqk, "bvb": bvb,
            "maska": maska, "maskb": mb_g1 if g else mb_g0,
        })

    nc = _get_nc()
    res = run_bass_kernel_spmd(nc, in_maps, core_ids=list(range(8)),
                               trace=_trace, **_run_kwargs)
    LAST_RESULT = res

    out = np.zeros((B, T, H), dtype=np.float32)
    for core in range(8):
        b = core // 2
        out[b][perms[core][:NQ * P]] = res.results[core]["out"]
    return out


# revision 18
# speedup vs baseline: 1.1967x; 1.1967x over previous
"""Single-head causal attention (B=4, T=2048, D=1024, H=64) on 8 TRN2 cores.

Sharding: core = (batch b, group g). Each core owns the interleaved half of
the query blocks of one batch (g=0: even 128-row blocks, g=1: odd), arranged
"mine first, partner second" via a host-side row permutation so the causal
structure (and hence the instruction stream) is identical on all 8 cores.

v2 structure (vs the v1 baseline):
- Wk and 8*Wq are packed into one [128]-wide stationary operand, so each
  projection pass computes k (partitions 0:63) and q (64:127) together:
  4 passes/chunk (v, hi*xh, lo*xh, hi*xl) instead of 7.
- Scores in 2 matmul passes instead of 3: pass 1 contracts 128 partitions
  with lhsT=[q_lo; q_hi] stacked and rhs=[k_hi; k_hi] duplicated
  (= q_hi*k_hi + q_lo*k_hi), pass 2 adds q_hi*k_lo (64-contraction).
- Per-block AV + finalize pipelined behind each score block (no monolithic
  AV tail), q/k split peeled per 512-col region so early blocks start while
  late regions still project, junk matmuls warm the PE during the x DMA.
- PSUM: 5-slot rotation shared by qk-region accumulators and score chunks
  (5 banks) + two-column-pass v (2 banks) + av/tr rotation (1 bank).
"""

import numpy as np
import ml_dtypes

import concourse.bass as bass
import concourse.bacc as bacc
import concourse.tile as tile
import concourse.mybir as mybir
from concourse.bass_utils import run_bass_kernel_spmd
from concourse.masks import make_identity

BF16 = mybir.dt.bfloat16
F32 = mybir.dt.float32
BFNP = ml_dtypes.bfloat16

B, T, D, H, P = 4, 2048, 1024, 64, 128
NB = T // P       # 16 key tiles (128 rows each) per batch
NQ = NB // 2      # 8 local query blocks per core
DC = D // P       # 8 contraction chunks
KC = 512          # matmul moving-dim chunk (one PSUM bank of fp32)
NEG = -1.0e30
ACT = mybir.ActivationFunctionType
ADD = mybir.AluOpType.add
SUB = mybir.AluOpType.subtract
MAX = mybir.AluOpType.max


def build_nc():
    nc = bacc.Bacc("TRN2", target_bir_lowering=False, debug=False,
                   num_devices=8)
    xt_hi = nc.dram_tensor("xt_hi", [D, T], BF16, kind="ExternalInput")
    xt_lo = nc.dram_tensor("xt_lo", [D, T], BF16, kind="ExternalInput")
    # weights pre-arranged on host to the SBUF layout [P, DC*{128,64}];
    # wkq = [Wk | 8*Wq] so one matmul emits k on partitions 0:63, q on 64:127
    wkq_hi = nc.dram_tensor("wkq_hi", [P, DC * P], BF16, kind="ExternalInput")
    wkq_lo = nc.dram_tensor("wkq_lo", [P, DC * P], BF16, kind="ExternalInput")
    wv = nc.dram_tensor("wv", [P, DC * H], BF16, kind="ExternalInput")
    bqk = nc.dram_tensor("bqk", [P, 1], F32, kind="ExternalInput")
    bvb = nc.dram_tensor("bvb", [H, 1], F32, kind="ExternalInput")
    maska = nc.dram_tensor("maska", [P, P], F32, kind="ExternalInput")
    maskb = nc.dram_tensor("maskb", [P, P], F32, kind="ExternalInput")
    out = nc.dram_tensor("out", [NQ * P, H], F32, kind="ExternalOutput")

    with tile.TileContext(nc) as tc:
        with (
            tc.tile_pool(name="singles", bufs=1) as singles,
            tc.tile_pool(name="probs", bufs=3) as probs_pool,
            tc.tile_pool(name="stats", bufs=6) as stats,
        ):
            # ---- persistent SBUF ----
            s_xth = singles.tile([P, DC, T], BF16)
            s_xtl = singles.tile([P, DC, T], BF16)
            s_wkqh = singles.tile([P, DC, P], BF16)
            s_wkql = singles.tile([P, DC, P], BF16)
            s_wv = singles.tile([P, DC, H], BF16)
            s_bqk = singles.tile([P, 1], F32)
            s_bv = singles.tile([H, 1], F32)
            s_ma = singles.tile([P, P], F32)
            s_mb = singles.tile([P, P], F32)

            # weights/bias/masks early on the scalar queue; x chunks follow
            # in consumption order, split per half-chunk so the first
            # projection matmuls start ~1us in: hi on sync, lo on scalar.
            for s_w, d_w in ((s_wv, wv), (s_wkqh, wkq_hi), (s_wkql, wkq_lo)):
                nc.scalar.dma_start(
                    s_w[:, :, :].rearrange("p c h -> p (c h)"), d_w[:, :])
            # bias/masks ride the (otherwise idle) gpsimd SWDGE queue so they
            # don't delay the x_lo stream behind them on the scalar ring
            nc.gpsimd.dma_start(s_bqk[:, :], bqk[:, :])
            nc.gpsimd.dma_start(s_bv[:, :], bvb[:, :])
            nc.gpsimd.dma_start(s_ma[:, :], maska[:, :])
            nc.gpsimd.dma_start(s_mb[:, :], maskb[:, :])
            TH = T // 2
            for c in range(DC):
                for h in range(2):
                    nc.sync.dma_start(
                        s_xth[:, c, h * TH:(h + 1) * TH],
                        xt_hi[c * P:(c + 1) * P, h * TH:(h + 1) * TH])
                    nc.scalar.dma_start(
                        s_xtl[:, c, h * TH:(h + 1) * TH],
                        xt_lo[c * P:(c + 1) * P, h * TH:(h + 1) * TH])

            s_v = singles.tile([H, T], BF16)
            s_vnat = singles.tile([P, NB, H], BF16)
            s_f = singles.tile([P, T], F32)       # split staging, per region
            s_k2 = singles.tile([P, T], BF16)     # [k_hi; k_hi] duplicated
            s_kl = singles.tile([P, T], BF16)     # k_lo on partitions 64:127
            s_q2 = singles.tile([P, NQ * P], BF16)  # [q_lo; q_hi] stacked
            s_probsT = singles.tile([P, NB, NQ * P], BF16)
            s_z = singles.tile([P, NQ, 4], F32)
            nc.gpsimd.memset(s_z[:, :, :], 0.0)
            s_outT = singles.tile([H, NQ * P], F32)
            s_id = singles.tile([H, H], F32)
            make_identity(nc, s_id[:, :])
            s_zs = singles.tile([P, NQ], F32)
            s_rz = singles.tile([P, NQ], F32)
            s_on = singles.tile([P, NQ, H], F32)
            # DRAM view matching s_on's [p, j, h] layout for a single store
            out_pjh = out[:, :].rearrange("(j p) h -> p j h", p=P)

            with (
                tc.tile_pool(name="work_psum", bufs=5, space="PSUM") as wk,
                tc.tile_pool(name="v_psum", bufs=1, space="PSUM") as vp,
                tc.tile_pool(name="av_psum", bufs=1, space="PSUM") as avp,
            ):
                # qk region accumulators take the first 4 of the 5 "w" slots;
                # score chunks (and the tail transposes) rotate through the
                # rest for the whole phase 2.
                qk_r = [wk.tile([P, KC], F32, tag="w", name=f"qk_r{r}")
                        for r in range(4)]
                v0 = vp.tile([H, KC], F32, tag="v")  # cols 0:512
                avps = avp.tile([H, NQ * P], F32, tag="av")

                # ---- phase 1: projections, chunks 0..DC-2 ----
                # per chunk: v cols 0:512, then [k|q]hi*xh, lo*xh, hi*xl
                for c in range(DC - 1):
                    nc.tensor.matmul(
                        v0[:, :], lhsT=s_wv[:, c, :],
                        rhs=s_xth[:, c, 0:KC],
                        start=(c == 0), stop=False)
                    for ip, (wpl, xpl) in enumerate(
                            ((s_wkqh, s_xth), (s_wkql, s_xth),
                             (s_wkqh, s_xtl))):
                        for r in range(4):
                            nc.tensor.matmul(
                                qk_r[r][:, :], lhsT=wpl[:, c, :],
                                rhs=xpl[:, c, r * KC:(r + 1) * KC],
                                start=(c == 0 and ip == 0), stop=False)

                # ---- peeled last chunk, per 512-col region + fused split ----
                c = DC - 1

                def peel_region(r):
                    n0, n1 = r * KC, (r + 1) * KC
                    if r == 0:
                        nc.tensor.matmul(v0[:, :], lhsT=s_wv[:, c, :],
                                         rhs=s_xth[:, c, 0:KC],
                                         start=False, stop=True)
                    for ip, (wpl, xpl) in enumerate(
                            ((s_wkqh, s_xth), (s_wkql, s_xth),
                             (s_wkqh, s_xtl))):
                        nc.tensor.matmul(qk_r[r][:, :], lhsT=wpl[:, c, :],
                                         rhs=xpl[:, c, n0:n1],
                                         start=False, stop=(ip == 2))
                    # split: f32 stage (+bias), k hi dup, k lo, q hi/lo stack
                    nc.scalar.activation(s_f[:, n0:n1], qk_r[r][:, :],
                                         ACT.Identity, bias=s_bqk[:, 0:1],
                                         scale=1.0)
                    if r == 0:
                        nc.scalar.copy(s_v[:, 0:KC], v0[:, :])
                        nc.sync.dma_start(s_vnat[:, 0:4, :], s_v[:, 0:KC],
                                          transpose=True)
                    nc.scalar.copy(s_k2[0:H, n0:n1], s_f[0:H, n0:n1])
                    nc.vector.tensor_scalar_mul(s_k2[H:P, n0:n1],
                                                s_f[0:H, n0:n1], 1.0)
                    nc.vector.tensor_tensor(s_kl[H:P, n0:n1], s_f[0:H, n0:n1],
                                            s_k2[0:H, n0:n1], SUB)
                    if r < 2:  # q lives in cols 0:1024 (partitions 64:127)
                        nc.scalar.copy(s_q2[H:P, n0:n1], s_f[H:P, n0:n1])
                        nc.vector.tensor_tensor(s_q2[0:H, n0:n1],
                                                s_f[H:P, n0:n1],
                                                s_q2[H:P, n0:n1], SUB)

                def score_block(i):
                    """scores+softmax+transpose for local q block i."""
                    K = P * (i + 1)
                    q0, q1 = i * P, (i + 1) * P
                    nch = (K + KC - 1) // KC
                    mx4 = stats.tile([P, 4], F32, tag="mx")
                    chunks = []
                    for pi, (koff, msk) in enumerate(((0, s_ma),
                                                      (NQ * P, s_mb))):
                        for n0 in range(0, K, KC):
                            nn = min(KC, K - n0)
                            sp = wk.tile([P, KC], F32, tag="w", name="sp")
                            # pass 1: [q_lo;q_hi].T @ [k_hi;k_hi]
                            nc.tensor.matmul(
                                sp[:, 0:nn], lhsT=s_q2[:, q0:q1],
                                rhs=s_k2[:, koff + n0:koff + n0 + nn],
                                start=True, stop=False)
                            # pass 2: q_hi.T @ k_lo (rows 64:127)
                            nc.tensor.matmul(
                                sp[:, 0:nn], lhsT=s_q2[H:P, q0:q1],
                                rhs=s_kl[H:P, koff + n0:koff + n0 + nn],
                                start=False, stop=True)
                            if n0 + nn == K:  # mask rides in the last chunk
                                nc.vector.tensor_tensor(
                                    sp[:, nn - P:nn], sp[:, nn - P:nn],
                                    msk[:, :], ADD)
                            ci = pi * nch + n0 // KC
                            nc.vector.tensor_reduce(
                                mx4[:, ci:ci + 1], sp[:, 0:nn],
                                axis=mybir.AxisListType.X, op=MAX)
                            chunks.append((sp, pi, n0, nn, ci))
                    negm = stats.tile([P, 1], F32, tag="negm")
                    nc.vector.tensor_reduce(negm[:, :], mx4[:, 0:2 * nch],
                                            axis=mybir.AxisListType.X,
                                            op=MAX, negate=True)
                    probs = probs_pool.tile([P, T], BF16)
                    for sp, pi, n0, nn, ci in chunks:
                        po = pi * K + n0
                        nc.scalar.activation(probs[:, po:po + nn], sp[:, 0:nn],
                                             ACT.Exp, bias=negm[:, :],
                                             scale=1.0,
                                             accum_out=s_z[:, i, ci:ci + 1])
                    # probsT middle index ti enumerates [a0..ai, b0..bi];
                    # part a transposes on sync, part b on scalar
                    nc.sync.dma_start(s_probsT[:, 0:i + 1, q0:q1],
                                      probs[:, 0:K], transpose=True)
                    nc.scalar.dma_start(s_probsT[:, i + 1:2 * (i + 1), q0:q1],
                                        probs[:, K:2 * K], transpose=True)

                def av_block(i):
                    """AV accumulation for local q block i (no finalize)."""
                    q0, q1 = i * P, (i + 1) * P
                    tiles = list(range(0, i + 1)) + list(range(NQ, NQ + i + 1))
                    for ti, t in enumerate(tiles):
                        nc.tensor.matmul(
                            avps[:, q0:q1], lhsT=s_vnat[:, t, :],
                            rhs=s_probsT[:, ti, q0:q1],
                            start=(ti == 0), stop=(ti == len(tiles) - 1))

                def v1_region(r):
                    """v columns of region r in {1,2,3}, all chunks; copy out
                    and transpose that region to natural layout."""
                    n0, n1 = r * KC, (r + 1) * KC
                    vr = vp.tile([H, KC], F32, tag="v", name=f"v_r{r}")
                    for cc in range(DC):
                        nc.tensor.matmul(
                            vr[:, :], lhsT=s_wv[:, cc, :],
                            rhs=s_xth[:, cc, n0:n1],
                            start=(cc == 0), stop=(cc == DC - 1))
                    nc.scalar.copy(s_v[:, n0:n1], vr[:, :])
                    nc.sync.dma_start(s_vnat[:, 4 * r:4 * r + 4, :],
                                      s_v[:, n0:n1], transpose=True)

                # ---- phase 2 schedule: peel regions / early blocks /
                # late blocks with per-block AV pipelined into the score
                # stat/exp bubbles; all finalize batched in the tail ----
                # NB: block i part a reads k regions 0..(K-1)//KC, part b
                # reads regions 2..2+(K-1)//KC, q block i reads region i//4;
                # av block i needs v regions 0..i//4 and 2..2+i//4.
                peel_region(0)
                peel_region(2)
                score_block(3)
                peel_region(1)
                score_block(2)
                peel_region(3)
                v1_region(1)
                score_block(1)
                v1_region(2)
                score_block(0)
                score_block(7)
                av_block(3)
                score_block(6)
                v1_region(3)
                av_block(2)
                av_block(1)
                score_block(5)
                av_block(0)
                score_block(4)
                av_block(7)
                av_block(6)
                av_block(5)
                av_block(4)

                # ---- tail: Z, reciprocal, +bv, transpose, scale, store ----
                nc.vector.tensor_reduce(s_zs[:, :], s_z[:, :, :],
                                        axis=mybir.AxisListType.X, op=ADD)
                nc.vector.reciprocal(s_rz[:, :], s_zs[:, :])
                nc.scalar.activation(s_outT[:, :], avps[:, :], ACT.Identity,
                                     bias=s_bv[:, 0:1], scale=1.0)
                for j in range(NQ):
                    tps = wk.tile([P, H], F32, tag="w", name="tps")
                    nc.tensor.transpose(tps[:, :], s_outT[:, j * P:(j + 1) * P],
                                        s_id[:, :])
                    nc.vector.tensor_scalar_mul(s_on[:, j, :], tps[:, :],
                                                s_rz[:, j:j + 1])
                nc.sync.dma_start(out_pjh[:, :, :], s_on[:, :, :])
    nc.compile()
    return nc


_NC_CACHE = {}


def _get_nc():
    if "nc" not in _NC_CACHE:
        _NC_CACHE["nc"] = build_nc()
    return _NC_CACHE["nc"]


def _split_bf(a):
    hi = a.astype(BFNP)
    lo = (a - hi.astype(np.float32)).astype(BFNP)
    return hi, lo


def _w_layout(w):
    # [D, M] -> SBUF layout [P, DC*M] (chunk-major along free dim)
    m = w.shape[1]
    return np.ascontiguousarray(
        w.reshape(DC, P, m).transpose(1, 0, 2).reshape(P, DC * m))


LAST_RESULT = None


def kernel(x, Wq, bq, Wk, bk, Wv, bv, _trace=False, **_run_kwargs):
    global LAST_RESULT
    x = np.ascontiguousarray(np.asarray(x, dtype=np.float32))
    Wq = np.asarray(Wq, dtype=np.float32)
    Wk = np.asarray(Wk, dtype=np.float32)
    Wv = np.asarray(Wv, dtype=np.float32)
    bq = np.asarray(bq, dtype=np.float32)
    bk = np.asarray(bk, dtype=np.float32)
    bv = np.asarray(bv, dtype=np.float32)

    # pack [Wk | 8*Wq] -> [D, 128]; the 8x is the softmax scale sqrt(H)
    wkq = np.concatenate([Wk, 8.0 * Wq], axis=1)
    wkqh, wkql = _split_bf(wkq)
    wkqh, wkql = _w_layout(wkqh), _w_layout(wkql)
    wvh = _w_layout(Wv.astype(BFNP))
    bqk = np.ascontiguousarray(
        np.concatenate([bk, 8.0 * bq]).reshape(P, 1))
    bvb = np.ascontiguousarray(bv.reshape(H, 1))
    r = np.arange(P)
    maska = np.where(r[None, :] <= r[:, None], 0.0, NEG).astype(np.float32)
    mb_g0 = np.full((P, P), NEG, dtype=np.float32)
    mb_g1 = np.zeros((P, P), dtype=np.float32)

    in_maps = []
    perms = []
    for core in range(8):
        b, g = core // 2, core % 2
        mine = list(range(g, NB, 2))
        partner = list(range(1 - g, NB, 2))
        perm = np.concatenate(
            [np.arange(blk * P, (blk + 1) * P) for blk in mine + partner])
        perms.append(perm)
        xt = np.ascontiguousarray(x[b][perm].T)  # [D, T] fp32
        xth, xtl = _split_bf(xt)
        in_maps.append({
            "xt_hi": xth, "xt_lo": xtl,
            "wkq_hi": wkqh, "wkq_lo": wkql, "wv": wvh,
            "bqk": bqk, "bvb": bvb,
            "maska": maska, "maskb": mb_g1 if g else mb_g0,
        })

    nc = _get_nc()
    res = run_bass_kernel_spmd(nc, in_maps, core_ids=list(range(8)),
                               trace=_trace, **_run_kwargs)
    LAST_RESULT = res

    out = np.zeros((B, T, H), dtype=np.float32)
    for core in range(8):
        b = core // 2
        out[b][perms[core][:NQ * P]] = res.results[core]["out"]
    return out
